# revision 13
# baseline (speedup 1.0000x reference)
"""Trainium2 Bass kernel for nn_AggregatorSubLayer (GNN message passing).

  out[r] = relu( concat(rev[r], user[uidx[r]], item[iidx[r]]) @ W )

Strategy (8 NeuronCores, data-parallel over the 500K review rows):
  - each core owns 62500 rows (padded to 62976 = 123 blocks of 512)
  - review rows are host-transposed to [128, R] so they feed the PE
    directly as the moving operand (features on partitions)
  - user/item lookups run on-device via the SWDGE dma_gather
    instruction (int16 indices). Indices are compacted per half-core
    (np.unique -> < 32768 unique rows) and the matching compact tables
    are shipped per core; the index tensor is pre-wrapped into the
    [128, R/16] layout the 8 GPSIMD cores expect.
  - gathered [row, feat] tiles are PE-transposed (identity matmul) to
    [feat, row]; three accumulating matmuls (one per 128-feature piece)
    produce outT[o, r] in PSUM; relu on the scalar engine; PE-transpose
    back; store natural [r, o].
"""

import os
import sys
import types

# the NEFF runs through PJRT on the axon TRN backend; a CPU pin (used by
# some harnesses for the jax reference) would break device dispatch
if os.environ.get("JAX_PLATFORMS") == "cpu" and "jax" not in sys.modules:
    del os.environ["JAX_PLATFORMS"]

sys.path.insert(0, "/opt/trn_rl_repo")

from contextlib import ExitStack

import numpy as np

import concourse.bass as bass
import concourse.bacc as bacc
import concourse.tile as tile
from concourse import bass_utils, mybir
from concourse.masks import make_identity

P = 128
D = 128
BLK = 512
TPB = BLK // P
GCHUNK = 4096

N_CORES = 8
N_REVIEWS = 500000
ROWS_PER_CORE = (N_REVIEWS + N_CORES - 1) // N_CORES  # 62500
R_PAD = ((ROWS_PER_CORE + BLK - 1) // BLK) * BLK      # 62976
CAP = 32768                                           # compact table capacity

MM_MODE = os.environ.get("AGG_MM_MODE", "f32")        # "f32" (exact) | "f32r"
NUM_SWDGE_QUEUES = int(os.environ.get("AGG_NQ", "4"))

F32 = mybir.dt.float32
F32R = mybir.dt.float32r
I16 = mybir.dt.int16
RELU = mybir.ActivationFunctionType.Relu

_last_exec_time_ns = None


def _install_ntff_hook():
    """The slim agent image lacks antenv.axon_hooks; recreate it so
    trace=True can capture NTFF profiles. No-op if unavailable."""
    try:
        import antenv
        from trn_agent_boot.trn_boot import _ntff_profile_via_ctypes

        if "antenv.axon_hooks" in sys.modules:
            return
        mod = types.ModuleType("antenv.axon_hooks")
        _h = {}
        mod.set_axon_ntff_profile_hook = lambda h: _h.__setitem__("h", h)
        mod.get_axon_ntff_profile_hook = lambda: _h.get("h")
        sys.modules["antenv.axon_hooks"] = mod
        antenv.axon_hooks = mod
        mod.set_axon_ntff_profile_hook(
            _ntff_profile_via_ctypes("/opt/axon/libaxon_pjrt.so")
        )
    except Exception:
        pass


def _plan_chunks(nb0, nb1):
    chunks = []
    for half, nb in ((0, nb0), (1, nb1)):
        base = 0 if half == 0 else nb0 * BLK
        rows = nb * BLK
        off = 0
        while off < rows:
            n = min(GCHUNK, rows - off)
            chunks.append((base + off, n, half))
            off += n
    return chunks


def _build_kernel(R, cap, mm_mode):
    assert R % BLK == 0
    NB = R // BLK
    nb0 = NB // 2
    nb1 = NB - nb0
    assert nb0 * BLK <= 32768 and nb1 * BLK <= 32768

    nc = bacc.Bacc(
        "TRN2",
        target_bir_lowering=False,
        debug=False,
        enable_asserts=False,
        num_swdge_queues=NUM_SWDGE_QUEUES,
    )

    # in "f32t" mode tables/stage tiles are declared float32r so the PE can
    # run the (bit-preserving) transposes in the faster f32r datapath; the
    # gather is a byte mover, so values stay exact fp32 bits
    TAB_DT = F32R if mm_mode == "f32t" else F32
    revT = nc.dram_tensor("revT", [P, R], F32, kind="ExternalInput").ap()
    uidx = nc.dram_tensor("uidx", [P, R // 16], I16, kind="ExternalInput").ap()
    iidx = nc.dram_tensor("iidx", [P, R // 16], I16, kind="ExternalInput").ap()
    utab0 = nc.dram_tensor("utab0", [cap, D], TAB_DT, kind="ExternalInput").ap()
    utab1 = nc.dram_tensor("utab1", [cap, D], TAB_DT, kind="ExternalInput").ap()
    itab0 = nc.dram_tensor("itab0", [cap, D], TAB_DT, kind="ExternalInput").ap()
    itab1 = nc.dram_tensor("itab1", [cap, D], TAB_DT, kind="ExternalInput").ap()
    w = nc.dram_tensor("w", [3 * D, D], F32, kind="ExternalInput").ap()
    out = nc.dram_tensor("out", [R, D], F32, kind="ExternalOutput").ap()

    utabs = (utab0, utab1)
    itabs = (itab0, itab1)
    chunks = _plan_chunks(nb0, nb1)

    with tile.TileContext(nc) as tc, ExitStack() as ctx:
        singles = ctx.enter_context(tc.tile_pool(name="singles", bufs=1))
        rev_pool = ctx.enter_context(tc.tile_pool(name="rev", bufs=3))
        stage_pool = ctx.enter_context(tc.tile_pool(name="stage", bufs=3))
        xt_pool = ctx.enter_context(tc.tile_pool(name="xt", bufs=4))
        out_pool = ctx.enter_context(tc.tile_pool(name="outp", bufs=3))
        if mm_mode == "f32r":
            tpsum = ctx.enter_context(tc.tile_pool(name="tpsum", bufs=3, space="PSUM"))
            opsum = ctx.enter_context(tc.tile_pool(name="opsum", bufs=2, space="PSUM"))
            otpsum = ctx.enter_context(
                tc.tile_pool(name="otpsum", bufs=3, space="PSUM")
            )
        else:
            tpsum = ctx.enter_context(tc.tile_pool(name="tpsum", bufs=4, space="PSUM"))
            opsum = ctx.enter_context(tc.tile_pool(name="opsum", bufs=4, space="PSUM"))

        w_sb = singles.tile([P, 3, D], F32)
        for j in range(3):
            nc.sync.dma_start(out=w_sb[:, j, :], in_=w[j * D : (j + 1) * D, :])
        if mm_mode == "f32r":
            # fp32r matmul operands must be produced by an instruction that
            # rounds to fp32r (BIR verifier rule) — DVE copy does the cast
            w_sbr = singles.tile([P, 3, D], F32R)
            nc.vector.tensor_copy(w_sbr[:], w_sb[:])
        uidx_sb = singles.tile([P, R // 16], I16)
        iidx_sb = singles.tile([P, R // 16], I16)
        nc.sync.dma_start(out=uidx_sb[:], in_=uidx[:])
        nc.sync.dma_start(out=iidx_sb[:], in_=iidx[:])
        ident = singles.tile([P, P], F32)
        make_identity(nc, ident[:])
        if mm_mode == "f32t":
            ident_r = singles.tile([P, P], F32R)
            nc.vector.tensor_copy(ident_r[:], ident[:])
            ident_t = ident_r
        else:
            ident_t = ident

        out_t = out.rearrange("(b n p) o -> b p n o", p=P, n=TPB)

        for row0, nrows, half in chunks:
            nblk = nrows // BLK
            usr_st = stage_pool.tile([P, GCHUNK // P, D], TAB_DT, tag="usr")
            itm_st = stage_pool.tile([P, GCHUNK // P, D], TAB_DT, tag="itm")
            nsplit = max(1, NUM_SWDGE_QUEUES // 2)
            for t_i, (st, tab_, idx_sb) in enumerate(
                ((usr_st, utabs[half], uidx_sb), (itm_st, itabs[half], iidx_sb))
            ):
                # sub-split each gather across queues for more outstanding
                # descriptors (the gather is HBM-latency bound per engine)
                bounds = [
                    row0 + (nrows * s // nsplit) // BLK * BLK for s in range(nsplit)
                ] + [row0 + nrows]
                for s in range(nsplit):
                    lo, hi = bounds[s], bounds[s + 1]
                    if lo >= hi:
                        continue
                    nc.gpsimd.dma_gather(
                        out_ap=st[:, (lo - row0) // P : (hi - row0) // P, :],
                        in_ap=tab_[:],
                        idxs_ap=idx_sb[:, lo // 16 : hi // 16],
                        num_idxs=hi - lo,
                        num_idxs_reg=hi - lo,
                        elem_size=D,
                        single_packet=False,
                        queue_num=(t_i * nsplit + s) % NUM_SWDGE_QUEUES,
                    )

            for j in range(nblk):
                b = row0 // BLK + j
                rev_sb = rev_pool.tile([P, BLK], F32)
                nc.sync.dma_start(out=rev_sb[:], in_=revT[:, b * BLK : (b + 1) * BLK])
                o_sb = out_pool.tile([P, TPB, D], F32)

                if mm_mode == "f32r":
                    uT = xt_pool.tile([P, BLK], F32R, tag="uT")
                    iT = xt_pool.tile([P, BLK], F32R, tag="iT")
                    for k in range(TPB):
                        for st, xT in ((usr_st, uT), (itm_st, iT)):
                            ps = tpsum.tile([P, P], F32, tag="t")
                            nc.tensor.transpose(ps[:], st[:, j * TPB + k, :], ident[:])
                            nc.vector.tensor_copy(xT[:, k * P : (k + 1) * P], ps[:])
                    rev_r = rev_pool.tile([P, BLK], F32R, tag="revr")
                    nc.scalar.copy(rev_r[:], rev_sb[:])
                    acc = otpsum.tile([P, BLK], F32, tag="acc")
                    for p_i, rhs in enumerate((rev_r, uT, iT)):
                        nc.tensor.matmul(
                            acc[:],
                            lhsT=w_sbr[:, p_i, :],
                            rhs=rhs[:],
                            start=(p_i == 0),
                            stop=(p_i == 2),
                        )
                    otr = xt_pool.tile([P, BLK], F32, tag="otr")
                    nc.scalar.activation(otr[:], acc[:], RELU)
                    for k in range(TPB):
                        ps = opsum.tile([P, P], F32, tag="o")
                        nc.tensor.transpose(ps[:], otr[:, k * P : (k + 1) * P], ident[:])
                        nc.vector.tensor_copy(o_sb[:, k, :], ps[:])
                else:
                    for k in range(TPB):
                        uT = xt_pool.tile([P, P], F32, tag="uT")
                        iT = xt_pool.tile([P, P], F32, tag="iT")
                        for st, xT in ((usr_st, uT), (itm_st, iT)):
                            ps = tpsum.tile([P, P], TAB_DT, tag="t")
                            nc.tensor.transpose(ps[:], st[:, j * TPB + k, :], ident_t[:])
                            nc.vector.tensor_copy(xT[:], ps[:])
                        acc = opsum.tile([P, P], F32, tag="acc")
                        for p_i, lhsT in enumerate(
                            (rev_sb[:, k * P : (k + 1) * P], uT[:], iT[:])
                        ):
                            nc.tensor.matmul(
                                acc[:],
                                lhsT=lhsT,
                                rhs=w_sb[:, p_i, :],
                                start=(p_i == 0),
                                stop=(p_i == 2),
                            )
                        nc.scalar.activation(o_sb[:, k, :], acc[:], RELU)

                nc.sync.dma_start(out=out_t[b], in_=o_sb[:])

    return nc


def _wrap_idx16(idx, R):
    w16 = idx.astype(np.int16).reshape(R // 16, 16).T
    return np.tile(w16, (8, 1))


def _prep_core_inputs(rev_rows, uidx_rows, iidx_rows, user_emb, item_emb, w):
    r0 = rev_rows.shape[0]
    R = R_PAD
    revT = np.zeros((P, R), dtype=np.float32)
    revT[:, :r0] = rev_rows.T
    uidx = np.zeros(R, dtype=np.int64)
    iidx = np.zeros(R, dtype=np.int64)
    uidx[:r0] = uidx_rows
    iidx[:r0] = iidx_rows

    NB = R // BLK
    split = (NB // 2) * BLK

    tabs = {}
    u16 = np.zeros(R, dtype=np.int64)
    i16 = np.zeros(R, dtype=np.int64)
    for half, sl in ((0, slice(0, split)), (1, slice(split, R))):
        uu, uinv = np.unique(uidx[sl], return_inverse=True)
        t = np.zeros((CAP, D), dtype=np.float32)
        t[: len(uu)] = user_emb[uu]
        tabs[f"utab{half}"] = t
        u16[sl] = uinv
        ii, iinv = np.unique(iidx[sl], return_inverse=True)
        t = np.zeros((CAP, D), dtype=np.float32)
        t[: len(ii)] = item_emb[ii]
        tabs[f"itab{half}"] = t
        i16[sl] = iinv

    return dict(
        revT=revT,
        uidx=_wrap_idx16(u16, R),
        iidx=_wrap_idx16(i16, R),
        w=np.ascontiguousarray(w, dtype=np.float32),
        **tabs,
    )


_nc_cache = {}


def kernel(
    review_embedding,
    item_embedding,
    user_embedding,
    adj_user_idx,
    adj_item_idx,
    agg_weights,
):
    global _last_exec_time_ns
    trace = os.environ.get("AGG_TRACE", "0") == "1"
    if trace:
        _install_ntff_hook()
        bass_utils.upload_artifacts = lambda tmpdir: f"file://{tmpdir}"

    key = (MM_MODE,)
    if key not in _nc_cache:
        nc = _build_kernel(R_PAD, CAP, MM_MODE)
        nc.compile()
        _nc_cache[key] = nc
    nc = _nc_cache[key]

    review_embedding = np.asarray(review_embedding, dtype=np.float32)
    item_embedding = np.asarray(item_embedding, dtype=np.float32)
    user_embedding = np.asarray(user_embedding, dtype=np.float32)
    adj_user_idx = np.asarray(adj_user_idx)
    adj_item_idx = np.asarray(adj_item_idx)
    agg_weights = np.asarray(agg_weights, dtype=np.float32)

    n = review_embedding.shape[0]
    in_maps = []
    for c in range(N_CORES):
        lo = c * ROWS_PER_CORE
        hi = min(lo + ROWS_PER_CORE, n)
        in_maps.append(
            _prep_core_inputs(
                review_embedding[lo:hi],
                adj_user_idx[lo:hi],
                adj_item_idx[lo:hi],
                user_embedding,
                item_embedding,
                agg_weights,
            )
        )

    res = bass_utils.run_bass_kernel_spmd(
        nc, in_maps, core_ids=list(range(N_CORES)), trace=trace
    )
    _last_exec_time_ns = res.exec_time_ns

    out = np.empty((n, D), dtype=np.float32)
    for c in range(N_CORES):
        lo = c * ROWS_PER_CORE
        hi = min(lo + ROWS_PER_CORE, n)
        out[lo:hi] = res.results[c]["out"][: hi - lo]
    return out
